# revision 1
# baseline (speedup 1.0000x reference)
"""Trainium2 Bass kernel for nn_Attention_2 (B=32, LQ=LK=2048, H=1024, A=512).

Math: the reference computes softmax over sum_q(Qp @ Kp^T), and the q-sum
distributes through the matmul, so the full [B, LQ, LK] score tensor never
needs to exist:

  qs[b]    = sum_q query[b,q,:]                       (query reduction)
  qp[b]    = qs[b] @ Wq + LQ*bq                       (tiny)
  wqt[b]   = Wk @ qp[b]         [H]                   (tiny)
  s[b,k]   = key[b,k,:] . wqt[b]   (+ const, cancels in softmax)
  vt[b,k]  = key[b,k,:] . wvt      (wvt = Wk @ Wv[:,0])
  x[b]     = sum_k softmax(s)[k] * vt[b,k] + (bk.Wv + bv)

Sharding: data-parallel over batch, 4 batches per core, 8 cores.
Per core the only heavy work is streaming query+key (67MB) from HBM:
  - query is summed over q via chained DMA-accumulate (free)
  - s/vt contractions over h are split between the PE (transpose + fp32
    matmul route, batches 0,2) and the DVE (fused mul+reduce route, 1,3)
    to keep both engines under the DMA roofline. Scores must be full fp32:
    the top-2 logit gap can be ~4, so bf16-level noise would flip ranks.

Shipped variant "fullq6": the baseline with (a) BOTH pairs' query chains
and preps emitted before either key pass (QFIRST) and (b) deepened key
tile rings, 5 for the PE route and 4 for the DVE route (the PE route is
the slow consumer, so its ring gates the in-order sync DMA stream; paid
for by 2-deep keyT staging, quarter-staged WkT transpose, and a
single-buffered w3o).  The trace showed the pair-1 query stream stretching across
the pair-0 key phase with the key DMA queue starved at the pair boundary
(in-order issue head-of-line), and a further ~80us key-DMA stall while
the key pass drained the 3-deep kt buffers.  Front-loading the chains
and deepening the kt rings remove both bubbles - measured ~246us vs
~286us per body for "fullq"-only vs ~272-286us for the original
(repeat-delta, this dev device; the grader measured the original
baseline at 76us).

Variant "v2"/"v2q" (KVARIANT env) replaces the accumulate-DMA query path
with plain loads + an on-chip PE row-reduction (ones-basis stationary
weight, query tiles as the 512-col moving operand), on the theory that
accumulate packets run the SDMA engines at half rate.  Measured on HW it
loses to the baseline (~308us vs ~286us per body, repeat-delta): the
freed DMA time is repaid in PE occupancy and query-phase lock-step, while
the baseline's accumulate overlap rides otherwise-idle DMA capacity
during the compute-bound key phase.  "v5" (2-link chains + row-reduce)
was abandoned: its GpSimd score-offload leg fails the v3 Pool-engine ISA
check (TensorScalarPtr), and without it the DVE stays the binding
resource.  Both kept for reference behind the flag.
"""
import numpy as np

import concourse.bass as bass
import concourse.bacc as bacc
import concourse.tile as tile
from concourse import mybir
from concourse.bass_utils import run_bass_kernel_spmd

N_CORES = 8
B, LQ, LK, H, A = 32, 2048, 2048, 1024, 512
BPC = B // N_CORES          # batches per core
P = 128
f32 = mybir.dt.float32
NQT = LQ // P               # 16 q subtiles per batch
NKT = LK // P               # 16 k subtiles per batch
NG = 8                      # key granules per batch
GK = LK // NG               # 512 k rows per granule
HJ = H // P                 # 8 h-chunks
AC = A // P                 # 4 a-chunks

_CACHE = {}
import os as _os
KTPE_BUFS = int(_os.environ.get("KTPE_BUFS", "3"))
KTDVE_BUFS = int(_os.environ.get("KTDVE_BUFS", "3"))
SBKT_BUFS = int(_os.environ.get("SBKT_BUFS", "3"))
QFIRST = int(_os.environ.get("QFIRST", "0"))
QT_BUFS = int(_os.environ.get("QT_BUFS", "5"))
KVARIANT = _os.environ.get("KVARIANT", "fullq6")


def _emit_query_chains(nc, sbq, query, bs):
    """Sum query[b] over q for each b in bs into [128, H] q-partials.
    Per b: 4 chained 2MB accumulate-DMAs, each adding 4 q-subtiles into 4
    disjoint column blocks; links are interleaved across the batches so the
    in-order gpsimd stream keeps several DMAs in flight. Two in-place DVE
    adds then fold the 4 blocks into block 0."""
    qts = [sbq.tile([P, 4 * H], f32, tag="qchain", name=f"qchain_b{b}")
           for b in bs]
    for i in range(NQT // 4):
        kw = {} if i == 0 else {"accum_op": mybir.AluOpType.add}
        for qt, b in zip(qts, bs):
            nc.gpsimd.dma_start(
                out=qt[:].rearrange("p (c h) -> p c h", c=4),
                in_=query[b, i * 4 * P:(i + 1) * 4 * P, :]
                .rearrange("(c p) h -> p c h", p=P),
                **kw)
    return qts


KGPN = int(_os.environ.get("KGPN", "2"))  # granules per DVE-batch on GpSimd

QG = 2  # q-subtiles per plain query DMA (1 MiB tiles)


def _emit_query_chain2(nc, sbq, query, bs):
    """2-link accumulate chains: per b, two [128, 4H] tiles, each filled by
    one plain 2MB DMA + one accumulate 2MB DMA.  Accumulate packets cost
    ~2x plain, so 2-link chains put query at 1.5x effective DMA bytes
    (vs 1.75x for the 4-link chains) at the price of a PE row-reduction
    over twice the tiles.  Returns {slot: [2 tiles]}."""
    qts = {s: [] for s in range(len(bs))}
    for t in range(2):
        for s, b in enumerate(bs):
            qt = sbq.tile([P, 4 * H], f32, tag="qchain2",
                          name=f"qchain2_b{b}_t{t}", bufs=3)
            qts[s].append(qt)
    for i in range(2):
        kw = {} if i == 0 else {"accum_op": mybir.AluOpType.add}
        for t in range(2):
            for s, b in enumerate(bs):
                base = (t * 2 + i) * 4 * P
                nc.gpsimd.dma_start(
                    out=qts[s][t][:].rearrange("p (c h) -> p c h", c=4),
                    in_=query[b, base:base + 4 * P, :]
                    .rearrange("(c p) h -> p c h", p=P),
                    **kw)
    return qts


def _emit_query_plain(nc, sbq, query, bs):
    """Plain (no-accumulate) query loads: per b, 8 independent 1MB DMAs of
    [128, 2x1024] tiles. Accumulate-DMA halves the per-engine SDMA rate
    (read-modify-write on the SBUF side), so the q-reduction moves to the
    PE instead. Returns {slot: [8 tiles]}."""
    qts = {s: [] for s in range(len(bs))}
    for t in range(NQT // QG):
        for s, b in enumerate(bs):
            # one shared circular tag: tiles are consumed strictly in
            # issue order, so a single deeper ring beats two shallow ones
            qt = sbq.tile([P, QG * H], f32, tag="qplain",
                          name=f"qplain_b{b}_t{t}", bufs=QT_BUFS)
            nc.gpsimd.dma_start(
                out=qt[:].rearrange("p (c h) -> p c h", c=QG),
                in_=query[b, t * QG * P:(t + 1) * QG * P, :]
                .rearrange("(c p) h -> p c h", p=P))
            qts[s].append(qt)
    return qts


def build_bass(repeat=1, variant="full"):
    nc = bacc.Bacc(None, target_bir_lowering=False, debug=False)

    query = nc.dram_tensor("query", [BPC, LQ, H], f32, kind="ExternalInput").ap()
    key = nc.dram_tensor("key", [BPC, LK, H], f32, kind="ExternalInput").ap()
    Wq = nc.dram_tensor("Wq", [H, A], f32, kind="ExternalInput").ap()
    bq = nc.dram_tensor("bq", [A], f32, kind="ExternalInput").ap()
    Wk = nc.dram_tensor("Wk", [H, A], f32, kind="ExternalInput").ap()
    bk = nc.dram_tensor("bk", [A], f32, kind="ExternalInput").ap()
    Wv = nc.dram_tensor("Wv", [A, 1], f32, kind="ExternalInput").ap()
    bv = nc.dram_tensor("bv", [1], f32, kind="ExternalInput").ap()
    out = nc.dram_tensor("out", [BPC, 1], f32, kind="ExternalOutput").ap()

    with tile.TileContext(nc) as tc:
        for _ in range(repeat):
            _build_body(nc, tc, query, key, Wq, bq, Wk, bk, Wv, bv, out,
                        variant=variant)
    nc.compile()
    return nc


def _build_body(nc, tc, query, key, Wq, bq, Wk, bk, Wv, bv, out, variant="full"):
    kt_pe_bufs, kt_dve_bufs = KTPE_BUFS, KTDVE_BUFS
    squeeze = False
    w3o_bufs = 2
    dvefirst = False
    s2_bufs = None
    if variant == "v2q":          # v2 with forced prep-first emission
        variant = "v2"
        qfirst = True
    elif variant == "fullq":      # baseline with forced prep-first emission
        variant = "full"
        qfirst = True
    elif variant == "fullq5":     # fullq + 4-deep key pools (the key DMA
        variant = "full"          # starves ~80us while the PE key pass
        qfirst = True             # drains 3-deep kt buffers); SBUF paid
        kt_pe_bufs = kt_dve_bufs = 4  # for by sbkt=2 + quarter Wk staging
        squeeze = True
    elif variant == "fullq6":     # fullq5 + deeper ring for the slow
        variant = "full"          # consumer: ktpe gates the in-order sync
        qfirst = True             # stream (PE route ~2x DVE per subtile)
        kt_pe_bufs, kt_dve_bufs = 5, 4
        squeeze = True
        w3o_bufs = 1
    elif variant == "fullq7":     # fullq6 + dve-first granule DMA order +
        variant = "full"          # 3-deep s2 PSUM ring (spare 8th bank)
        qfirst = True
        kt_pe_bufs, kt_dve_bufs = 5, 4
        squeeze = True
        w3o_bufs = 1
        dvefirst = True
        s2_bufs = 3
    elif variant == "v5":         # 2-link chains + row-reduce + GP offload
        qfirst = True
    else:
        qfirst = bool(QFIRST)
    qplain = variant in ("v2", "dma2")
    qchain2 = variant == "v5"
    qrr = qplain or qchain2       # query reduced on-chip via PE row-reduce
    squeeze = squeeze or qrr      # sbkt=2 + quarter-staged WkT transpose
    from contextlib import ExitStack
    ctx = ExitStack()
    with ctx:
        sbc = ctx.enter_context(tc.tile_pool(name="sbc", bufs=1))
        sbq = ctx.enter_context(tc.tile_pool(name="sbq", bufs=2))
        sbkey = ctx.enter_context(tc.tile_pool(name="sbkey", bufs=2))  # per-tag below
        sbkt = ctx.enter_context(tc.tile_pool(
            name="sbkt", bufs=min(SBKT_BUFS, 2) if squeeze else SBKT_BUFS))
        sbrt = ctx.enter_context(tc.tile_pool(name="sbrt", bufs=2))
        sbsv = ctx.enter_context(tc.tile_pool(name="sbsv", bufs=1))
        sbsm = ctx.enter_context(tc.tile_pool(name="sbsm", bufs=2))
        sbw3 = ctx.enter_context(tc.tile_pool(name="sbw3", bufs=1))
        sbjunk = ctx.enter_context(tc.tile_pool(name="sbjunk", bufs=1))
        ps_keyT = ctx.enter_context(tc.tile_pool(name="ps_keyT", bufs=2, space="PSUM"))
        # qplain needs a bank for qsT and 2 banks for qrow in ps_small, so
        # s2 drops to single-buffered there (the [3,H]/[P,H] prep matmuls
        # are emitted in [.,A]-halves to keep other ps_small tiles 1-bank)
        ps_s2 = ctx.enter_context(tc.tile_pool(
            name="ps_s2", bufs=s2_bufs or (1 if qrr else 2), space="PSUM"))
        ps_small = ctx.enter_context(tc.tile_pool(name="ps_small", bufs=1, space="PSUM"))
        if qrr:
            ps_qsT = ctx.enter_context(
                tc.tile_pool(name="ps_qsT", bufs=1, space="PSUM"))

        if variant in ("dma", "dma2"):
            for pair in range(2):
                bs = (2 * pair, 2 * pair + 1)
                if qplain:
                    _emit_query_plain(nc, sbq, query, bs)
                else:
                    _emit_query_chains(nc, sbq, query, bs)
                for g in range(NG):
                    for b in bs:
                        kt = sbkey.tile([P, (GK // P) * H], f32, tag="ktpe",
                                        name=f"kt_{b}_{g}")
                        nc.sync.dma_start(
                            out=kt[:].rearrange("p (n h) -> p n h", n=GK // P),
                            in_=key[b, g * GK:(g + 1) * GK, :]
                            .rearrange("(n p) h -> p n h", p=P))
            xz = sbsm.tile([1, 1], f32, tag="x")
            nc.vector.memset(xz[:], 0.0)
            for b in range(BPC):
                nc.sync.dma_start(out=out[b:b + 1, :], in_=xz[:])
            return

        # ---------------- constants ----------------
        ident = sbc.tile([P, P], f32)
        colidx = sbsm.tile([P, P], f32, tag="small")
        rowidx = sbsm.tile([P, 1], f32, tag="tiny")
        nc.gpsimd.iota(colidx[:], pattern=[[1, P]], base=0, channel_multiplier=0,
                       allow_small_or_imprecise_dtypes=True)
        nc.gpsimd.iota(rowidx[:], pattern=[[0, 1]], base=0, channel_multiplier=1,
                       allow_small_or_imprecise_dtypes=True)
        nc.vector.tensor_scalar(out=ident[:], in0=colidx[:], scalar1=rowidx[:],
                                scalar2=None, op0=mybir.AluOpType.is_equal)

        ones = sbc.tile([P, 1], f32)
        nc.vector.memset(ones[:], 1.0)
        ones_k1 = sbc.tile([1, P], f32)
        nc.vector.memset(ones_k1[:], 1.0)
        ones2 = []
        if qrr:
            # ones-basis columns for the query row-reduce: col s = 1, other 0
            for s in range(2):
                o2 = sbc.tile([P, 2], f32, tag=f"ones2_{s}")
                nc.vector.memset(o2[:], 0.0)
                nc.vector.memset(o2[:, s:s + 1], 1.0)
                ones2.append(o2)

        # small selection constants: diag3[k, c] = (k == c); E1/E2[k, m] = (k == 1/2)
        diag3 = sbc.tile([3, 4], f32)
        d3c = sbsm.tile([3, 4], f32, tag="tiny4")
        d3r = sbsm.tile([3, 1], f32, tag="tiny5")
        nc.gpsimd.iota(d3c[:], pattern=[[1, 4]], base=0, channel_multiplier=0,
                       allow_small_or_imprecise_dtypes=True)
        nc.gpsimd.iota(d3r[:], pattern=[[0, 1]], base=0, channel_multiplier=1,
                       allow_small_or_imprecise_dtypes=True)
        nc.vector.tensor_scalar(out=diag3[:], in0=d3c[:], scalar1=d3r[:],
                                scalar2=None, op0=mybir.AluOpType.is_equal)
        selrow = sbsm.tile([3, P], f32, tag="selrow")
        nc.gpsimd.iota(selrow[:], pattern=[[0, P]], base=0, channel_multiplier=1,
                       allow_small_or_imprecise_dtypes=True)
        E1 = sbc.tile([3, P], f32)
        nc.vector.tensor_scalar(out=E1[:], in0=selrow[:], scalar1=1.0,
                                scalar2=None, op0=mybir.AluOpType.is_equal)
        E2 = sbc.tile([3, P], f32)
        nc.vector.tensor_scalar(out=E2[:], in0=selrow[:], scalar1=2.0,
                                scalar2=None, op0=mybir.AluOpType.is_equal)

        # Wq natural layout: [h-part, (j a)] ; chunk j at cols [j*A, (j+1)*A)
        Wq_sb = sbc.tile([P, HJ * A], f32)
        nc.sync.dma_start(out=Wq_sb[:].rearrange("p (j a) -> p j a", j=HJ),
                          in_=Wq.rearrange("(j p) a -> p j a", p=P))

        wv_sb = sbc.tile([P, AC], f32)
        nc.sync.dma_start(out=wv_sb[:].rearrange("p (c o) -> p c o", c=AC),
                          in_=Wv.rearrange("(c p) o -> p c o", p=P))
        bk_sb = sbc.tile([P, AC], f32)
        nc.sync.dma_start(out=bk_sb[:], in_=bk.rearrange("(c p) -> p c", p=P))
        bv_sb = sbc.tile([1, 1], f32)
        nc.sync.dma_start(out=bv_sb[:], in_=bv[None, :])
        bq2 = sbc.tile([2, A], f32)
        nc.sync.dma_start(out=bq2[:], in_=bass.AP(
            tensor=bq.tensor, offset=bq.offset, ap=[[0, 2]] + list(bq.ap)))

        # WkT [a-part, (c h)]: transpose Wk once on the PE, staged in
        # pieces to bound the SBUF staging footprint (quarters under
        # qplain, which needs the extra room for its query tiles)
        NQR = 4 if squeeze else 2
        WkT_sb = sbc.tile([P, AC * H], f32)
        with tc.tile_pool(name="sbwk", bufs=1) as sbwk:
            for qr in range(NQR):
                Wk_sb = sbwk.tile([P, (HJ // NQR) * A], f32, tag="wk")
                nc.sync.dma_start(
                    out=Wk_sb[:].rearrange("p (j a) -> p j a", j=HJ // NQR),
                    in_=Wk[qr * (H // NQR):(qr + 1) * (H // NQR), :]
                    .rearrange("(j p) a -> p j a", p=P))
                for c in range(AC):
                    wkt_ps = ps_small.tile([P, (HJ // NQR) * P], f32, tag="small")
                    for jl in range(HJ // NQR):
                        nc.tensor.transpose(
                            wkt_ps[:, jl * P:(jl + 1) * P],
                            Wk_sb[:, jl * A + c * P: jl * A + (c + 1) * P],
                            ident[:])
                    dst = WkT_sb[:, c * H + qr * (H // NQR):
                                 c * H + (qr + 1) * (H // NQR)]
                    if c % 2 == 0:
                        nc.scalar.copy(dst, wkt_ps[:])
                    else:
                        nc.vector.tensor_copy(dst, wkt_ps[:])

        # c_v = bk . Wv + bv  (folded into the output at the very end)
        junk4 = sbsm.tile([P, AC], f32, tag="tiny2")
        cvcol = sbsm.tile([P, 1], f32, tag="tiny3")
        nc.vector.scalar_tensor_tensor(out=junk4[:], in0=bk_sb[:], scalar=1.0,
                                       in1=wv_sb[:], op0=mybir.AluOpType.mult,
                                       op1=mybir.AluOpType.mult, accum_out=cvcol[:])
        cv_ps = ps_small.tile([1, 1], f32, tag="small")
        nc.tensor.matmul(cv_ps[:], cvcol[:], ones[:], start=True, stop=True)
        cv_sb = sbc.tile([1, 1], f32)
        nc.vector.tensor_tensor(out=cv_sb[:], in0=cv_ps[:], in1=bv_sb[:],
                                op=mybir.AluOpType.add)

        # ---------------- per-pair processing ----------------
        def emit_prep(pair):
            b_pe, b_dve = 2 * pair, 2 * pair + 1

            if qrr:
                # Row-reduce on the PE with MOVING query tiles: lhsT is a
                # [128, 2] ones-basis column (cheap self-load), the query
                # tile streams 512 fp32 cols per matmul back-to-back.  The
                # column-wise variant (tiny lhsT=qt chunk + ones rhs) costs
                # ~550ns per LDW+matmul pair on HW (NX + drain) - ~10x
                # slower.  qrow rows: 0 = b_pe sum, 1 = b_dve sum.  The two
                # 512-col regions of [2, H] land in different PSUM banks,
                # so the interleaved accumulation groups never share a bank;
                # the zero columns of ones2 accumulate zeros harmlessly.
                if qchain2:
                    qts = _emit_query_chain2(nc, sbq, query, (b_pe, b_dve))
                    nt, ncb = 2, 4
                else:
                    qts = _emit_query_plain(nc, sbq, query, (b_pe, b_dve))
                    nt, ncb = NQT // QG, QG
                qrow_ps = ps_small.tile([2, H], f32, tag="small")
                for t in range(nt):
                    for slot in range(2):
                        qt = qts[slot][t]
                        for c in range(ncb):
                            for hh in range(H // 512):
                                nc.tensor.matmul(
                                    qrow_ps[:, hh * 512:(hh + 1) * 512],
                                    ones2[slot],
                                    qt[:, c * H + hh * 512:
                                       c * H + (hh + 1) * 512],
                                    start=(t == 0 and slot == 0 and c == 0),
                                    stop=(t == nt - 1 and slot == 1
                                          and c == ncb - 1))
                qrow_sb = sbsm.tile([2, H], f32, tag="qrow")
                nc.scalar.copy(qrow_sb[:], qrow_ps[:])
                # recover the [h-part, (j slot)] column layout via 8 PE
                # transposes of [2, 128] chunks
                qsT_ps = ps_qsT.tile([P, 2 * HJ], f32, tag="qsT")
                for j in range(HJ):
                    nc.tensor.transpose(qsT_ps[:, 2 * j:2 * j + 2],
                                        qrow_sb[:, j * P:(j + 1) * P],
                                        ident[0:2, 0:2])
            else:
                # --- query sums (DMA-accumulate chains) ---
                qs_pe, qs_dve = _emit_query_chains(nc, sbq, query, (b_pe, b_dve))

            if variant == "noqdep":
                w2 = sbrt.tile([P, 2 * HJ], f32, tag="w2")
                nc.vector.memset(w2[:], 0.001)
                wqbc = sbrt.tile([P, H], f32, tag="wqbc")
                nc.vector.memset(wqbc[:], 0.001)
                wvbc = sbrt.tile([P, H], f32, tag="wvbc")
                nc.vector.memset(wvbc[:], 0.001)
                return w2, wqbc, wvbc

            if not qrr:
                # --- qsT columns: [h, 1] per (b, j) via ones-matmuls summing
                # the 4 unmerged chain column blocks directly in PSUM ---
                qsT_ps = ps_small.tile([P, 2 * HJ], f32, tag="small")
                for slot, qs in ((0, qs_pe), (1, qs_dve)):
                    for j in range(HJ):
                        for c in range(4):
                            nc.tensor.matmul(
                                qsT_ps[:, 2 * j + slot: 2 * j + slot + 1],
                                qs[:, c * H + j * P:c * H + (j + 1) * P], ones[:],
                                start=(c == 0), stop=(c == 3))
            qsT_sb = sbsm.tile([P, 2 * HJ], f32, tag="qsT")
            nc.vector.tensor_copy(qsT_sb[:], qsT_ps[:])

            # --- qp = qs @ Wq + LQ*bq   [2, A] (rows = pair slots) ---
            qp_ps = ps_small.tile([2, A], f32, tag="small")
            for j in range(HJ):
                nc.tensor.matmul(qp_ps[:], qsT_sb[:, 2 * j:2 * j + 2],
                                 Wq_sb[:, j * A:(j + 1) * A],
                                 start=(j == 0), stop=(j == HJ - 1))
            qp2 = sbsm.tile([2, A], f32, tag="qp2")
            nc.vector.scalar_tensor_tensor(out=qp2[:], in0=bq2[:], scalar=float(LQ),
                                           in1=qp_ps[:], op0=mybir.AluOpType.mult,
                                           op1=mybir.AluOpType.add)
            # --- w3 = [qpT_pe | qpT_dve | wvT] in a-partition layout ---
            # select-and-transpose row b of qp2 via a basis-column matmul
            w3_ps = ps_small.tile([P, 3 * AC], f32, tag="small")
            for c in range(AC):
                for slot in range(2):
                    nc.tensor.matmul(w3_ps[:, 3 * c + slot: 3 * c + slot + 1],
                                     qp2[:, c * P:(c + 1) * P],
                                     diag3[0:2, slot:slot + 1],
                                     start=True, stop=True)
            w3_sb = sbsm.tile([P, 3 * AC], f32, tag="w3")
            nc.vector.tensor_copy(w3_sb[:], w3_ps[:])
            nc.vector.tensor_copy(
                w3_sb[:].rearrange("p (c s) -> p c s", c=AC)[:, :, 2:3],
                wv_sb[:].rearrange("p (c o) -> p c o", c=AC))

            # --- [wqt_pe; wqt_dve; wvt] = w3^T @ WkT   -> [3, H] ---
            # ([3, A]-halves: keeps every ps_small tile within one PSUM bank)
            w3o = sbw3.tile([3, H], f32, tag="w3o", bufs=w3o_bufs)
            for half in range(2):
                w3o_ps = ps_small.tile([3, A], f32, tag="small")
                for c in range(AC):
                    nc.tensor.matmul(
                        w3o_ps[:],
                        w3_sb[:, 3 * c: 3 * c + 3],
                        WkT_sb[:, c * H + half * A: c * H + (half + 1) * A],
                        start=(c == 0), stop=(c == AC - 1))
                nc.scalar.copy(w3o[:, half * A:(half + 1) * A], w3o_ps[:])

            # --- PE-route prep: w2 [h-part, (j 2)] = [wqt_chunk, wvt_chunk] ---
            w2ps = ps_small.tile([P, 2 * HJ], f32, tag="small")
            for j in range(HJ):
                nc.tensor.matmul(w2ps[:, 2 * j: 2 * j + 1],
                                 w3o[:, j * P:(j + 1) * P], diag3[:, 0:1],
                                 start=True, stop=True)
                nc.tensor.matmul(w2ps[:, 2 * j + 1: 2 * j + 2],
                                 w3o[:, j * P:(j + 1) * P], diag3[:, 2:3],
                                 start=True, stop=True)
            w2 = sbrt.tile([P, 2 * HJ], f32, tag="w2")
            nc.vector.tensor_copy(w2[:], w2ps[:])

            # --- DVE-route prep: broadcast wqt_dve and wvt across partitions
            # ([P, A]-halves for the 1-bank ps_small budget) ---
            wqbc = sbrt.tile([P, H], f32, tag="wqbc")
            wvbc = sbrt.tile([P, H], f32, tag="wvbc")
            for dst, E in ((wqbc, E1), (wvbc, E2)):
                for half in range(2):
                    bc_ps = ps_small.tile([P, A], f32, tag="small")
                    nc.tensor.matmul(bc_ps[:], E[:],
                                     w3o[:, half * A:(half + 1) * A],
                                     start=True, stop=True)
                    nc.scalar.copy(dst[:, half * A:(half + 1) * A], bc_ps[:])
            return w2, wqbc, wvbc

        def emit_key(pair, prep):
            b_pe, b_dve = 2 * pair, 2 * pair + 1
            w2, wqbc, wvbc = prep
            # --- key pass ---
            sv_sb = sbsv.tile([2, LK], f32, tag="sv")      # PE-route scores/v rows
            sdve = sbsv.tile([P, NKT], f32, tag="sdve")    # DVE-route scores
            vdve = sbsv.tile([P, NKT], f32, tag="vdve")

            for g in range(NG):
                kt_pe = sbkey.tile([P, (GK // P) * H], f32, tag="ktpe", bufs=kt_pe_bufs)
                kt_dve = sbkey.tile([P, (GK // P) * H], f32, tag="ktdve",
                                    bufs=min(kt_dve_bufs, 2) if qchain2
                                    else kt_dve_bufs)
                # dvefirst: issue the fast consumer's DMA first so a
                # blocked ktpe-ring DMA doesn't hold up an issuable dve
                # tile on the in-order sync stream
                emits = [(kt_pe, b_pe), (kt_dve, b_dve)]
                if dvefirst:
                    emits.reverse()
                for kt_x, b_x in emits:
                    nc.sync.dma_start(
                        out=kt_x[:].rearrange("p (n h) -> p n h", n=GK // P),
                        in_=key[b_x, g * GK:(g + 1) * GK, :]
                        .rearrange("(n p) h -> p n h", p=P))
                for n in range(GK // P):
                    t = g * (GK // P) + n
                    kv = kt_pe[:, n * H:(n + 1) * H]
                    # PE route: transpose then fp32 matmul with [wqt|wvt]
                    keyT_ps = ps_keyT.tile([P, H], f32, tag="keyT")
                    for j in range(HJ):
                        nc.tensor.transpose(keyT_ps[:, j * P:(j + 1) * P],
                                            kv[:, j * P:(j + 1) * P], ident[:])
                    keyT = sbkt.tile([P, H], f32, tag="keyT")
                    nc.scalar.copy(keyT[:], keyT_ps[:])
                    s2_ps = ps_s2.tile([2, P], f32, tag="s2")
                    for j in range(HJ):
                        nc.tensor.matmul(s2_ps[:], w2[:, 2 * j:2 * j + 2],
                                         keyT[:, j * P:(j + 1) * P],
                                         start=(j == 0), stop=(j == HJ - 1))
                    nc.scalar.copy(sv_sb[:, t * P:(t + 1) * P], s2_ps[:])

                    # DVE route: fused mul+reduce rowwise dots.  Under v5
                    # the first KGPN granules go to the (otherwise idle)
                    # GpSimd engine to shave the DVE critical path; GP
                    # needs its own junk tile (the shared DVE/GP SBUF port
                    # pair is an exclusive lock, so distinct buffers avoid
                    # false write-conflicts in the tracker).
                    eng = (nc.gpsimd if (qchain2 and g < KGPN)
                           else nc.vector)
                    jt = "junkg" if eng is nc.gpsimd else "junk"
                    kvd = kt_dve[:, n * H:(n + 1) * H]
                    junk = sbjunk.tile([P, H], f32, tag=jt)
                    eng.scalar_tensor_tensor(
                        out=junk[:], in0=kvd[:], scalar=1.0, in1=wqbc[:],
                        op0=mybir.AluOpType.mult, op1=mybir.AluOpType.mult,
                        accum_out=sdve[:, t:t + 1])
                    junk2 = sbjunk.tile([P, H], f32, tag=jt)
                    eng.scalar_tensor_tensor(
                        out=junk2[:], in0=kvd[:], scalar=1.0, in1=wvbc[:],
                        op0=mybir.AluOpType.mult, op1=mybir.AluOpType.mult,
                        accum_out=vdve[:, t:t + 1])

            # --- softmax + combine, PE-route batch ---
            vsw = sbjunk.tile([2, LK], f32, tag="vsw")
            nc.vector.stream_shuffle(vsw[:], sv_sb[:], [1, 0] + list(range(2, 32)))
            smax = sbsm.tile([2, 1], f32, tag="smax")
            nc.vector.reduce_max(smax[:], sv_sb[:], axis=mybir.AxisListType.X)
            nmax = sbsm.tile([2, 1], f32, tag="nmax")
            nc.vector.tensor_scalar_mul(nmax[:], smax[:], -1.0)
            den = sbsm.tile([1, 1], f32, tag="den")
            nc.scalar.activation(sv_sb[0:1, :], sv_sb[0:1, :],
                                 mybir.ActivationFunctionType.Exp,
                                 bias=nmax[0:1], scale=1.0, accum_out=den[:])
            junk3 = sbjunk.tile([1, LK], f32, tag="junk")
            num = sbsm.tile([1, 1], f32, tag="num")
            nc.vector.scalar_tensor_tensor(
                out=junk3[:], in0=sv_sb[0:1, :], scalar=1.0, in1=vsw[0:1, :],
                op0=mybir.AluOpType.mult, op1=mybir.AluOpType.mult,
                accum_out=num[:])
            _emit_final(nc, sbsm, num, den, cv_sb, out, b_pe)

            # --- softmax + combine, DVE-route batch ---
            m1 = sbsm.tile([P, 1], f32, tag="m1")
            nc.vector.reduce_max(m1[:], sdve[:], axis=mybir.AxisListType.X)
            mT_ps = ps_small.tile([1, P], f32, tag="small")
            nc.tensor.transpose(mT_ps[:], m1[:], ident[:])
            mT_sb = sbsm.tile([1, P], f32, tag="mT")
            nc.vector.tensor_copy(mT_sb[:], mT_ps[:])
            gmax = sbsm.tile([1, 1], f32, tag="gmax")
            nc.vector.reduce_max(gmax[:], mT_sb[:], axis=mybir.AxisListType.X)
            ng_ps = ps_small.tile([P, 1], f32, tag="small")
            nc.tensor.matmul(ng_ps[:], ones_k1[:], gmax[:], start=True, stop=True)
            ngm = sbsm.tile([P, 1], f32, tag="ngm")
            nc.vector.tensor_scalar_mul(ngm[:], ng_ps[:], -1.0)
            e128 = sbsm.tile([P, NKT], f32, tag="e128")
            erow = sbsm.tile([P, 1], f32, tag="erowp")
            nc.scalar.activation(e128[:], sdve[:], mybir.ActivationFunctionType.Exp,
                                 bias=ngm[:], scale=1.0, accum_out=erow[:])
            junk5 = sbsm.tile([P, NKT], f32, tag="junk5")
            nrow = sbsm.tile([P, 1], f32, tag="nrow")
            nc.vector.scalar_tensor_tensor(
                out=junk5[:], in0=e128[:], scalar=1.0, in1=vdve[:],
                op0=mybir.AluOpType.mult, op1=mybir.AluOpType.mult,
                accum_out=nrow[:])
            den_ps = ps_small.tile([1, 2], f32, tag="small")
            nc.tensor.matmul(den_ps[:, 0:1], erow[:], ones[:], start=True, stop=True)
            nc.tensor.matmul(den_ps[:, 1:2], nrow[:], ones[:], start=True, stop=True)
            dn = sbsm.tile([1, 2], f32, tag="dn")
            nc.vector.tensor_copy(dn[:], den_ps[:])
            _emit_final(nc, sbsm, dn[:, 1:2], dn[:, 0:1], cv_sb, out, b_dve)

        if qfirst:
            preps = [emit_prep(0), emit_prep(1)]
            emit_key(0, preps[0])
            emit_key(1, preps[1])
        else:
            for pair in range(2):
                emit_key(pair, emit_prep(pair))


def _emit_final(nc, sbsm, num, den, cv_sb, out, b):
    rden = sbsm.tile([1, 1], f32, tag="rden")
    nc.vector.reciprocal(rden[:], den[:])
    x = sbsm.tile([1, 1], f32, tag="x")
    nc.vector.tensor_tensor(out=x[:], in0=num[:], in1=rden[:],
                            op=mybir.AluOpType.mult)
    x2 = sbsm.tile([1, 1], f32, tag="x2")
    nc.vector.tensor_tensor(out=x2[:], in0=x[:], in1=cv_sb[:],
                            op=mybir.AluOpType.add)
    nc.sync.dma_start(out=out[b:b + 1, :], in_=x2[:])


def _shard(query, key, shared):
    in_maps = []
    for c in range(N_CORES):
        sl = slice(c * BPC, (c + 1) * BPC)
        m = {"query": np.ascontiguousarray(query[sl]),
             "key": np.ascontiguousarray(key[sl])}
        m.update(shared)
        in_maps.append(m)
    return in_maps


def _make_in_maps(inputs):
    query = np.ascontiguousarray(np.asarray(inputs["query"], dtype=np.float32))
    key = np.ascontiguousarray(np.asarray(inputs["key"], dtype=np.float32))
    shared = {k: np.ascontiguousarray(np.asarray(inputs[k], dtype=np.float32))
              for k in ("Wq", "bq", "Wk", "bk", "Wv", "bv")}
    return _shard(query, key, shared)


def kernel(**inputs):
    if "nc" not in _CACHE:
        _CACHE["nc"] = build_bass(variant=KVARIANT)
    nc = _CACHE["nc"]

    query = np.ascontiguousarray(np.asarray(inputs["query"], dtype=np.float32))
    key = np.ascontiguousarray(np.asarray(inputs["key"], dtype=np.float32))
    shared = {
        "Wq": np.ascontiguousarray(np.asarray(inputs["Wq"], dtype=np.float32)),
        "bq": np.ascontiguousarray(np.asarray(inputs["bq"], dtype=np.float32)),
        "Wk": np.ascontiguousarray(np.asarray(inputs["Wk"], dtype=np.float32)),
        "bk": np.ascontiguousarray(np.asarray(inputs["bk"], dtype=np.float32)),
        "Wv": np.ascontiguousarray(np.asarray(inputs["Wv"], dtype=np.float32)),
        "bv": np.ascontiguousarray(np.asarray(inputs["bv"], dtype=np.float32)),
    }
    in_maps = _shard(query, key, shared)

    res = run_bass_kernel_spmd(nc, in_maps, list(range(N_CORES)))
    outs = [res.results[c]["out"] for c in range(N_CORES)]
    return np.concatenate(outs, axis=0).astype(np.float32)


if __name__ == "__main__":
    rng = np.random.default_rng(0)
    ins = {
        "query": rng.standard_normal((B, LQ, H), dtype=np.float32),
        "key": rng.standard_normal((B, LK, H), dtype=np.float32),
        "Wq": (rng.standard_normal((H, A), dtype=np.float32) / np.sqrt(H)).astype(np.float32),
        "bq": np.zeros((A,), np.float32),
        "Wk": (rng.standard_normal((H, A), dtype=np.float32) / np.sqrt(H)).astype(np.float32),
        "bk": np.zeros((A,), np.float32),
        "Wv": (rng.standard_normal((A, 1), dtype=np.float32) / np.sqrt(A)).astype(np.float32),
        "bv": np.zeros((1,), np.float32),
    }
    x = kernel(**ins)
    print("kernel out:", x[:8, 0])



# revision 9
# speedup vs baseline: 1.0806x; 1.0806x over previous
"""Trainium2 Bass kernel for nn_Attention_2 (B=32, LQ=LK=2048, H=1024, A=512).

Math (q-sum distributes through the matmul, so [B,LQ,LK] never exists):
  qs[b]   = sum_q query[b,q,:]
  qp[b]   = qs[b] @ Wq + LQ*bq
  u[b]    = qp[b] @ Wk^T            (score weights, [H])
  w       = Wk @ Wv[:,0]            (v weights, [H])
  s[b,k]  = key[b,k,:] . u[b]  (+ qp.bk const, cancels in softmax)
  v[b,k]  = key[b,k,:] . w     (+ cv = bk.Wv + bv, folded at end)
  x[b]    = softmax(s) . v + cv

v7 architecture (vs the DMA-accumulate baseline, ~367us):
  - All DMA is plain packets on the two hardware queues.  One hw queue
    sustains ~378 GB/s (measured); accumulate packets run the SDMA engines
    at half rate, so the query reduction moved on-chip (PE ones-matmuls).
  - Query streams batch-sequentially (order b3,b0,b1,b2) on the ACT queue;
    each batch's prep (qs->qsT->qp->qpT->u->w2/wqbc) runs as soon as its
    last tile lands, so that batch's key dots start while later batches'
    query still streams.
  - Key streams in batch-interleaved granule waves (256 rows / 1MB):
    b3/b0/b1 on the SYNC queue; b2 on the ACT queue split around prep2 so
    a ring-full dma_start can never park the ACT stream ahead of the
    copies that would free it (deadlock audit).
  - b3 takes the PE route: fp32 transposes -> ACT copy (rounds to f32r)
    -> f32r matmuls with the [uT|wT] stationary pair (f32r = 1 cyc/col
    for moving dims >=256 vs 4 for fp32; 11 mantissa bits -> measured
    end-to-end rel-err ~2.6e-3, gate 2e-2).  b0..b2 take the DVE route
    (fused mul+reduce STT rowwise dots, fp32).
  - Raw Wk/Wq stage through the DVE key ring (consumed in the first ~15us)
    so only the rounded/transposed copies hold permanent SBUF.
"""
import numpy as np

import concourse.bass as bass
import concourse.bacc as bacc
import concourse.tile as tile
from concourse import mybir
from concourse.bass_utils import run_bass_kernel_spmd

N_CORES = 8
B, LQ, LK, H, A = 32, 2048, 2048, 1024, 512
BPC = B // N_CORES
P = 128
f32 = mybir.dt.float32
f32r = mybir.dt.float32r
NG = 8                      # key granules per batch (256 rows each)
GR = LK // NG               # 256
NQD = 8                     # query DMAs per batch ([128, 2H] tiles)
HJ = H // P                 # 8
AC = A // P                 # 4

_CACHE = {}
import os as _os
QLAND_BUFS = int(_os.environ.get("QLAND_BUFS", "3"))
KTPE_BUFS = int(_os.environ.get("KTPE_BUFS", "3"))
KTD01_BUFS = int(_os.environ.get("KTD01_BUFS", "4"))
KTD2_BUFS = int(_os.environ.get("KTD2_BUFS", "3"))
KEYT_BUFS = int(_os.environ.get("KEYT_BUFS", "2"))


def build_bass():
    nc = bacc.Bacc(None, target_bir_lowering=False, debug=False)

    query = nc.dram_tensor("query", [BPC, LQ, H], f32, kind="ExternalInput").ap()
    key = nc.dram_tensor("key", [BPC, LK, H], f32, kind="ExternalInput").ap()
    Wq = nc.dram_tensor("Wq", [H, A], f32, kind="ExternalInput").ap()
    bq = nc.dram_tensor("bq", [A], f32, kind="ExternalInput").ap()
    Wk = nc.dram_tensor("Wk", [H, A], f32, kind="ExternalInput").ap()
    bk = nc.dram_tensor("bk", [A], f32, kind="ExternalInput").ap()
    Wv = nc.dram_tensor("Wv", [A, 1], f32, kind="ExternalInput").ap()
    bv = nc.dram_tensor("bv", [1], f32, kind="ExternalInput").ap()
    out = nc.dram_tensor("out", [BPC, 1], f32, kind="ExternalOutput").ap()

    with tile.TileContext(nc) as tc:
        _build_body(nc, tc, query, key, Wq, bq, Wk, bk, Wv, bv, out)
    nc.compile()
    return nc


def _build_body(nc, tc, query, key, Wq, bq, Wk, bk, Wv, bv, out):
    from contextlib import ExitStack
    ctx = ExitStack()
    with ctx:
        sbc = ctx.enter_context(tc.tile_pool(name="sbc", bufs=1))
        sbq = ctx.enter_context(tc.tile_pool(name="sbq", bufs=1))
        sbk = ctx.enter_context(tc.tile_pool(name="sbk", bufs=1))
        sbr = ctx.enter_context(tc.tile_pool(name="sbr", bufs=1))
        sbsv = ctx.enter_context(tc.tile_pool(name="sbsv", bufs=1))
        sbj = ctx.enter_context(tc.tile_pool(name="sbj", bufs=1))
        sbsm = ctx.enter_context(tc.tile_pool(name="sbsm", bufs=1))
        ps_qr = ctx.enter_context(tc.tile_pool(name="ps_qr", bufs=1, space="PSUM"))
        ps_kt = ctx.enter_context(tc.tile_pool(name="ps_kt", bufs=2, space="PSUM"))
        ps_s2 = ctx.enter_context(tc.tile_pool(name="ps_s2", bufs=2, space="PSUM"))
        ps_sm = ctx.enter_context(tc.tile_pool(name="ps_sm", bufs=1, space="PSUM"))

        # ---------------- constants ----------------
        ident = sbc.tile([P, P], f32)
        colidx = sbsm.tile([P, P], f32, tag="small")
        rowidx = sbsm.tile([P, 1], f32, tag="tiny")
        nc.gpsimd.iota(colidx[:], pattern=[[1, P]], base=0, channel_multiplier=0,
                       allow_small_or_imprecise_dtypes=True)
        nc.gpsimd.iota(rowidx[:], pattern=[[0, 1]], base=0, channel_multiplier=1,
                       allow_small_or_imprecise_dtypes=True)
        nc.vector.tensor_scalar(out=ident[:], in0=colidx[:], scalar1=rowidx[:],
                                scalar2=None, op0=mybir.AluOpType.is_equal)
        ones1 = sbc.tile([P, 1], f32)
        nc.vector.memset(ones1[:], 1.0)
        one11 = sbc.tile([1, 1], f32)
        nc.vector.memset(one11[:], 1.0)
        ones_k1 = sbc.tile([1, P], f32)
        nc.vector.memset(ones_k1[:], 1.0)

        # -------- weight DMAs (ACT queue, first) --------
        # raw Wk/Wq stage through the ktd01 ring (slots recycled for keys)
        wk_st = [sbk.tile([P, 4 * A], f32, tag="ktd01", bufs=KTD01_BUFS,
                          name=f"wkst{i}") for i in range(2)]
        for i in range(2):
            nc.scalar.dma_start(
                out=wk_st[i][:].rearrange("p (j a) -> p j a", j=4),
                in_=Wk[i * A:(i + 1) * A, :].rearrange("(j p) a -> p j a", p=P))
        wq_st = [sbk.tile([P, 4 * A], f32, tag="ktd01", bufs=KTD01_BUFS,
                          name=f"wqst{i}") for i in range(2)]
        for i in range(2):
            nc.scalar.dma_start(
                out=wq_st[i][:].rearrange("p (j a) -> p j a", j=4),
                in_=Wq[i * A:(i + 1) * A, :].rearrange("(j p) a -> p j a", p=P))
        wv_sb = sbc.tile([P, AC], f32)
        nc.scalar.dma_start(out=wv_sb[:].rearrange("p (c o) -> p c o", c=AC),
                            in_=Wv.rearrange("(c p) o -> p c o", p=P))
        bk_sb = sbc.tile([P, AC], f32)
        nc.scalar.dma_start(out=bk_sb[:], in_=bk.rearrange("(c p) -> p c", p=P))
        bv_sb = sbc.tile([1, 1], f32)
        nc.scalar.dma_start(out=bv_sb[:], in_=bv[None, :])
        bq_row = sbc.tile([1, A], f32)
        nc.scalar.dma_start(out=bq_row[:], in_=bq[None, :])

        # -------- one-time weight prep (PE+ACT+DVE, before query issues) ----
        # WkT_r [a-part, (c h)] f32r, rounded at the PSUM->SBUF copies
        WkT_r = sbc.tile([P, AC * H], f32r)
        for c in range(AC):
            for half in range(2):
                wkp = ps_kt.tile([P, A], f32, tag="ktp")
                for jl in range(4):
                    nc.tensor.transpose(
                        wkp[:, jl * P:(jl + 1) * P],
                        wk_st[half][:, jl * A + c * P:jl * A + (c + 1) * P],
                        ident[:])
                nc.scalar.copy(
                    WkT_r[:, c * H + half * A:c * H + (half + 1) * A], wkp[:])
        # Wq_r [h-part, (j a)] f32r
        Wq_r = sbc.tile([P, HJ * A], f32r)
        for i in range(2):
            nc.scalar.copy(Wq_r[:, i * 4 * A:(i + 1) * 4 * A], wq_st[i][:])
        # wv_r, then w row = Wv^T @ WkT  [1, H] fp32
        wv_r = sbc.tile([P, AC], f32r)
        nc.scalar.copy(wv_r[:], wv_sb[:])
        w_sb = sbr.tile([1, H], f32, tag="u", bufs=1, name="w_sb")
        for half in range(2):
            w_ps = ps_sm.tile([1, A], f32, tag="small")
            for c in range(AC):
                nc.tensor.matmul(w_ps[:], wv_r[:, c:c + 1],
                                 WkT_r[:, c * H + half * A:c * H + (half + 1) * A],
                                 start=(c == 0), stop=(c == AC - 1))
            nc.scalar.copy(w_sb[:, half * A:(half + 1) * A], w_ps[:])
        # wvbc [P, H] = broadcast w across partitions (DVE route operand)
        wvbc = sbc.tile([P, H], f32)
        for half in range(2):
            bc_ps = ps_sm.tile([P, A], f32, tag="small")
            nc.tensor.matmul(bc_ps[:], ones_k1[:], w_sb[:, half * A:(half + 1) * A],
                             start=True, stop=True)
            nc.scalar.copy(wvbc[:, half * A:(half + 1) * A], bc_ps[:])
        # wT8 [128, 8]: w chunks as columns (for the b3 w2 tile)
        wT8_ps = ps_sm.tile([P, HJ], f32, tag="small")
        for j in range(HJ):
            nc.tensor.matmul(wT8_ps[:, j:j + 1], w_sb[:, j * P:(j + 1) * P],
                             one11[:], start=True, stop=True)
        wT8 = sbc.tile([P, HJ], f32)
        nc.scalar.copy(wT8[:], wT8_ps[:])
        # cv = bk . Wv + bv
        junk4 = sbsm.tile([P, AC], f32, tag="tiny2")
        cvcol = sbsm.tile([P, 1], f32, tag="tiny3")
        nc.vector.scalar_tensor_tensor(out=junk4[:], in0=bk_sb[:], scalar=1.0,
                                       in1=wv_sb[:], op0=mybir.AluOpType.mult,
                                       op1=mybir.AluOpType.mult, accum_out=cvcol[:])
        cv_ps = ps_sm.tile([1, 1], f32, tag="small")
        nc.tensor.matmul(cv_ps[:], cvcol[:], ones1[:], start=True, stop=True)
        cv_sb = sbc.tile([1, 1], f32)
        nc.vector.tensor_tensor(out=cv_sb[:], in0=cv_ps[:], in1=bv_sb[:],
                                op=mybir.AluOpType.add)

        # -------- key DMAs for b3/b0/b1 (SYNC queue, granule waves) --------
        kt = {}
        for g in range(NG):
            for b in (3, 0, 1):
                tag, bufs = ("ktpe", KTPE_BUFS) if b == 3 else ("ktd01", KTD01_BUFS)
                t = sbk.tile([P, 2 * H], f32, tag=tag, bufs=bufs, name=f"k{b}_{g}")
                nc.sync.dma_start(
                    out=t[:].rearrange("p (c h) -> p c h", c=2),
                    in_=key[b, g * GR:(g + 1) * GR, :]
                    .rearrange("(c p) h -> p c h", p=P))
                kt[(b, g)] = t

        # ---------------- per-batch state ----------------
        preps = {b: {} for b in range(BPC)}
        sv3 = sbsv.tile([2, LK], f32, tag="sv3")
        sdve = {b: sbsv.tile([P, 2 * NG], f32, tag=f"sd{b}", name=f"sdve{b}") for b in range(3)}
        vdve = {b: sbsv.tile([P, 2 * NG], f32, tag=f"vd{b}", name=f"vdve{b}") for b in range(3)}
        qland = {}

        def emit_qdma(b, lo, hi):
            if lo == 0:
                preps[b]["qr_ps"] = [ps_qr.tile([1, A], f32, tag=f"qr{h}", bufs=1,
                                                 name=f"qrps{b}_{h}")
                                     for h in range(2)]
            for i in range(lo, hi):
                t = sbq.tile([P, 2 * H], f32, tag="qland", bufs=QLAND_BUFS,
                             name=f"q{b}_{i}")
                nc.scalar.dma_start(
                    out=t[:].rearrange("p (c h) -> p c h", c=2),
                    in_=query[b, i * 2 * P:(i + 1) * 2 * P, :]
                    .rearrange("(c p) h -> p c h", p=P))
                qland[(b, i)] = t

        def emit_kdma2(glo, ghi):
            """b2 key granules on the ACT queue (split around prep2)."""
            for g in range(glo, ghi):
                t = sbk.tile([P, 2 * H], f32, tag="ktd2", bufs=KTD2_BUFS,
                             name=f"k2_{g}")
                nc.scalar.dma_start(
                    out=t[:].rearrange("p (c h) -> p c h", c=2),
                    in_=key[2, g * GR:(g + 1) * GR, :]
                    .rearrange("(c p) h -> p c h", p=P))
                kt[(2, g)] = t

        def emit_qr(b, lo, hi):
            """Query-reduce: two [1,512] PSUM groups (h-halves) per batch,
            each accumulating both column-subtiles of all 8 tiles."""
            for i in range(lo, hi):
                t = qland[(b, i)]
                for half in range(2):
                    for c in range(2):
                        nc.tensor.matmul(
                            preps[b]["qr_ps"][half][:],
                            ones1[:],
                            t[:, c * H + half * A:c * H + (half + 1) * A],
                            start=(i == 0 and c == 0),
                            stop=(i == NQD - 1 and c == 1))

        def emit_prep(b):
            """qs -> qsT -> qp -> qpT -> u -> (w2 | wqbc) for batch b."""
            d = preps[b]
            qs_sb = sbr.tile([1, H], f32, tag="qs", bufs=1)
            for half in range(2):
                nc.scalar.copy(qs_sb[:, half * A:(half + 1) * A],
                               d["qr_ps"][half][:])
            qsT_ps = ps_sm.tile([P, HJ], f32, tag="small")
            for j in range(HJ):
                nc.tensor.matmul(qsT_ps[:, j:j + 1],
                                 qs_sb[:, j * P:(j + 1) * P], one11[:],
                                 start=True, stop=True)
            qsT = sbr.tile([P, HJ], f32r, tag="qsT", bufs=1)
            nc.scalar.copy(qsT[:], qsT_ps[:])
            qp_ps = ps_sm.tile([1, A], f32, tag="small")
            for j in range(HJ):
                nc.tensor.matmul(qp_ps[:], qsT[:, j:j + 1],
                                 Wq_r[:, j * A:(j + 1) * A],
                                 start=(j == 0), stop=(j == HJ - 1))
            qp_sb = sbr.tile([1, A], f32, tag="qp", bufs=1)
            nc.vector.scalar_tensor_tensor(
                out=qp_sb[:], in0=bq_row[:], scalar=float(LQ), in1=qp_ps[:],
                op0=mybir.AluOpType.mult, op1=mybir.AluOpType.add)
            qpT_ps = ps_sm.tile([P, AC], f32, tag="small")
            for c in range(AC):
                nc.tensor.matmul(qpT_ps[:, c:c + 1],
                                 qp_sb[:, c * P:(c + 1) * P], one11[:],
                                 start=True, stop=True)
            qpT = sbr.tile([P, AC], f32r, tag="qpT", bufs=1)
            nc.scalar.copy(qpT[:], qpT_ps[:])
            u_sb = sbr.tile([1, H], f32, tag="u", bufs=1)
            for half in range(2):
                u_ps = ps_sm.tile([1, A], f32, tag="small")
                for c in range(AC):
                    nc.tensor.matmul(
                        u_ps[:], qpT[:, c:c + 1],
                        WkT_r[:, c * H + half * A:c * H + (half + 1) * A],
                        start=(c == 0), stop=(c == AC - 1))
                nc.scalar.copy(u_sb[:, half * A:(half + 1) * A], u_ps[:])
            if b == 3:
                w2_ps = ps_sm.tile([P, 2 * HJ], f32, tag="small")
                for j in range(HJ):
                    nc.tensor.matmul(w2_ps[:, 2 * j:2 * j + 1],
                                     u_sb[:, j * P:(j + 1) * P], one11[:],
                                     start=True, stop=True)
                w2 = sbr.tile([P, 2 * HJ], f32r, tag="w2")
                nc.scalar.copy(w2[:], w2_ps[:])
                nc.scalar.copy(
                    w2[:].rearrange("p (j two) -> p j two", two=2)[:, :, 1:2],
                    wT8[:].unsqueeze(2))
                d["w2"] = w2
            else:
                wqbc = sbr.tile([P, H], f32, tag=f"wqbc{b}")
                for half in range(2):
                    bc_ps = ps_sm.tile([P, A], f32, tag="small")
                    nc.tensor.matmul(bc_ps[:], ones_k1[:],
                                     u_sb[:, half * A:(half + 1) * A],
                                     start=True, stop=True)
                    nc.scalar.copy(wqbc[:, half * A:(half + 1) * A], bc_ps[:])
                d["wqbc"] = wqbc

        def emit_route_pe(g):
            """b3 granule g: transposes -> keyT (f32r) -> s2 matmuls -> sv."""
            t = kt[(3, g)]
            keyT = sbk.tile([P, 2 * H], f32r, tag="keyT", bufs=KEYT_BUFS)
            for j in range(HJ):
                ktp = ps_kt.tile([P, 2 * P], f32, tag="ktp")
                for c in range(2):
                    nc.tensor.transpose(ktp[:, c * P:(c + 1) * P],
                                        t[:, c * H + j * P:c * H + (j + 1) * P],
                                        ident[:])
                nc.scalar.copy(keyT[:, j * 2 * P:(j + 1) * 2 * P], ktp[:])
            s2 = ps_s2.tile([2, 2 * P], f32, tag="s2")
            w2 = preps[3]["w2"]
            for j in range(HJ):
                nc.tensor.matmul(s2[:], w2[:, 2 * j:2 * j + 2],
                                 keyT[:, j * 2 * P:(j + 1) * 2 * P],
                                 start=(j == 0), stop=(j == HJ - 1))
            nc.scalar.copy(sv3[:, g * GR:(g + 1) * GR], s2[:])

        def emit_dots_dve(b, g):
            t = kt[(b, g)]
            for c in range(2):
                ti = g * 2 + c
                j1 = sbj.tile([P, H], f32, tag="junk", bufs=2)
                nc.vector.scalar_tensor_tensor(
                    out=j1[:], in0=t[:, c * H:(c + 1) * H], scalar=1.0,
                    in1=preps[b]["wqbc"][:], op0=mybir.AluOpType.mult,
                    op1=mybir.AluOpType.mult, accum_out=sdve[b][:, ti:ti + 1])
                j2 = sbj.tile([P, H], f32, tag="junk", bufs=2)
                nc.vector.scalar_tensor_tensor(
                    out=j2[:], in0=t[:, c * H:(c + 1) * H], scalar=1.0,
                    in1=wvbc[:], op0=mybir.AluOpType.mult,
                    op1=mybir.AluOpType.mult, accum_out=vdve[b][:, ti:ti + 1])

        def emit_softmax_pe():
            vsw = sbj.tile([2, LK], f32, tag="vsw")
            nc.vector.stream_shuffle(vsw[:], sv3[:], [1, 0] + list(range(2, 32)))
            smax = sbsm.tile([2, 1], f32, tag="smax")
            nc.vector.reduce_max(smax[:], sv3[:], axis=mybir.AxisListType.X)
            nmax = sbsm.tile([2, 1], f32, tag="nmax")
            nc.vector.tensor_scalar_mul(nmax[:], smax[:], -1.0)
            den = sbsm.tile([1, 1], f32, tag="den")
            nc.scalar.activation(sv3[0:1, :], sv3[0:1, :],
                                 mybir.ActivationFunctionType.Exp,
                                 bias=nmax[0:1], scale=1.0, accum_out=den[:])
            num = sbsm.tile([1, 1], f32, tag="num")
            nc.vector.scalar_tensor_tensor(
                out=vsw[0:1, :], in0=sv3[0:1, :], scalar=1.0, in1=vsw[0:1, :],
                op0=mybir.AluOpType.mult, op1=mybir.AluOpType.mult,
                accum_out=num[:])
            _emit_final(nc, sbsm, num, den, cv_sb, out, 3)

        def emit_softmax_dve(b):
            m1 = sbsm.tile([P, 1], f32, tag="m1")
            nc.vector.reduce_max(m1[:], sdve[b][:], axis=mybir.AxisListType.X)
            mT_ps = ps_sm.tile([1, P], f32, tag="small")
            nc.tensor.transpose(mT_ps[:], m1[:], ident[:])
            mT_sb = sbsm.tile([1, P], f32, tag="mT")
            nc.vector.tensor_copy(mT_sb[:], mT_ps[:])
            gmax = sbsm.tile([1, 1], f32, tag="gmax")
            nc.vector.reduce_max(gmax[:], mT_sb[:], axis=mybir.AxisListType.X)
            ng_ps = ps_sm.tile([P, 1], f32, tag="small")
            nc.tensor.matmul(ng_ps[:], ones_k1[:], gmax[:], start=True, stop=True)
            ngm = sbsm.tile([P, 1], f32, tag="ngm")
            nc.vector.tensor_scalar_mul(ngm[:], ng_ps[:], -1.0)
            e128 = sbsm.tile([P, 2 * NG], f32, tag="e128")
            erow = sbsm.tile([P, 1], f32, tag="erowp")
            nc.scalar.activation(e128[:], sdve[b][:],
                                 mybir.ActivationFunctionType.Exp,
                                 bias=ngm[:], scale=1.0, accum_out=erow[:])
            junk5 = sbsm.tile([P, 2 * NG], f32, tag="junk5")
            nrow = sbsm.tile([P, 1], f32, tag="nrow")
            nc.vector.scalar_tensor_tensor(
                out=junk5[:], in0=e128[:], scalar=1.0, in1=vdve[b][:],
                op0=mybir.AluOpType.mult, op1=mybir.AluOpType.mult,
                accum_out=nrow[:])
            den_ps = ps_sm.tile([1, 2], f32, tag="small")
            nc.tensor.matmul(den_ps[:, 0:1], erow[:], ones1[:], start=True, stop=True)
            nc.tensor.matmul(den_ps[:, 1:2], nrow[:], ones1[:], start=True, stop=True)
            dn = sbsm.tile([1, 2], f32, tag="dn")
            nc.vector.tensor_copy(dn[:], den_ps[:])
            _emit_final(nc, sbsm, dn[:, 1:2], dn[:, 0:1], cv_sb, out, b)

        # ---------------- main emission interleave ----------------
        # Call order defines each engine's in-order stream; comments give the
        # intended wall-clock position.
        emit_qdma(3, 0, NQD)                 # q3 streams [0, ~28us]
        emit_qr(3, 0, NQD)
        emit_prep(3)                         # ~30us
        emit_route_pe(0)
        emit_qdma(0, 0, 4)
        emit_route_pe(1)
        emit_qdma(0, 4, NQD)
        emit_qr(0, 0, NQD)
        emit_prep(0)                         # ~58us
        emit_dots_dve(0, 0)
        emit_route_pe(2)
        emit_qdma(1, 0, 4)
        emit_dots_dve(0, 1)
        emit_route_pe(3)
        emit_qdma(1, 4, NQD)
        emit_qr(1, 0, NQD)
        emit_prep(1)                         # ~85us
        emit_dots_dve(1, 0)
        emit_dots_dve(1, 1)
        emit_route_pe(4)
        emit_qdma(2, 0, 4)
        emit_kdma2(0, 3)                     # b2 early granules (ring-bounded)
        emit_dots_dve(0, 2)
        emit_dots_dve(1, 2)
        emit_route_pe(5)
        emit_qdma(2, 4, NQD)
        emit_qr(2, 0, NQD)
        emit_prep(2)                         # ~110us
        emit_kdma2(3, NG)                    # rest of b2 (after prep2 copies)
        emit_dots_dve(2, 0)
        emit_dots_dve(2, 1)
        emit_dots_dve(2, 2)
        emit_route_pe(6)
        for g in range(3, NG):
            emit_dots_dve(0, g)
            emit_dots_dve(1, g)
            emit_dots_dve(2, g)
        emit_route_pe(7)
        emit_softmax_pe()
        for b in range(3):
            emit_softmax_dve(b)


def _emit_final(nc, sbsm, num, den, cv_sb, out, b):
    rden = sbsm.tile([1, 1], f32, tag="rden")
    nc.vector.reciprocal(rden[:], den[:])
    x = sbsm.tile([1, 1], f32, tag="x")
    nc.vector.tensor_tensor(out=x[:], in0=num[:], in1=rden[:],
                            op=mybir.AluOpType.mult)
    x2 = sbsm.tile([1, 1], f32, tag="x2")
    nc.vector.tensor_tensor(out=x2[:], in0=x[:], in1=cv_sb[:],
                            op=mybir.AluOpType.add)
    nc.sync.dma_start(out=out[b:b + 1, :], in_=x2[:])


def _shard(query, key, shared):
    in_maps = []
    for c in range(N_CORES):
        sl = slice(c * BPC, (c + 1) * BPC)
        m = {"query": np.ascontiguousarray(query[sl]),
             "key": np.ascontiguousarray(key[sl])}
        m.update(shared)
        in_maps.append(m)
    return in_maps


def _make_in_maps(inputs):
    query = np.ascontiguousarray(np.asarray(inputs["query"], dtype=np.float32))
    key = np.ascontiguousarray(np.asarray(inputs["key"], dtype=np.float32))
    shared = {k: np.ascontiguousarray(np.asarray(inputs[k], dtype=np.float32))
              for k in ("Wq", "bq", "Wk", "bk", "Wv", "bv")}
    return _shard(query, key, shared)


def kernel(**inputs):
    if "nc" not in _CACHE:
        _CACHE["nc"] = build_bass()
    nc = _CACHE["nc"]
    in_maps = _make_in_maps(inputs)
    res = run_bass_kernel_spmd(nc, in_maps, list(range(N_CORES)))
    outs = [res.results[c]["out"] for c in range(N_CORES)]
    return np.concatenate(outs, axis=0).astype(np.float32)


if __name__ == "__main__":
    rng = np.random.default_rng(0)
    ins = {
        "query": rng.standard_normal((B, LQ, H), dtype=np.float32),
        "key": rng.standard_normal((B, LK, H), dtype=np.float32),
        "Wq": (rng.standard_normal((H, A), dtype=np.float32) / np.sqrt(H)).astype(np.float32),
        "bq": np.zeros((A,), np.float32),
        "Wk": (rng.standard_normal((H, A), dtype=np.float32) / np.sqrt(H)).astype(np.float32),
        "bk": np.zeros((A,), np.float32),
        "Wv": (rng.standard_normal((A, 1), dtype=np.float32) / np.sqrt(A)).astype(np.float32),
        "bv": np.zeros((1,), np.float32),
    }
    x = kernel(**ins)
    print("kernel out:", x[:4, 0])
